# revision 29
# baseline (speedup 1.0000x reference)
"""Trainium2 Bass kernel for GQA attention block (nn_Attention_20272245637793).

Reference computation (B=2, S=2048, H=2048, 16 q heads / 8 kv heads, D=128):
    q = hs @ Wq.T ; k = hs @ Wk.T ; v = hs @ Wv.T
    rope(q), rope(k); causal softmax(q k^T / sqrt(D)) @ v ; out @ Wo.T

Sharding (8 cores): core i = (b, g) with b = i // 4 (data-parallel over
batch), g = i % 4 (tensor-parallel over kv-head groups; kv heads {2g, 2g+1},
q heads {4g..4g+3}).  Each core computes 1/8 of every GEMM and a partial
o_proj over its 512 head-dims; the host sums the 4 partials per batch.

v3 changes over v2 (trace-driven; v2 = 280.5us, PE busy 84%):
  * Phase 2 was ScalarE-exp-bound: 96 ACTIVATEs x (cols*0.833ns + ~274ns
    fixed).  Scores psum tiles are now [128,1536] (3 banks) and the exp'd
    stripes are packed contiguously, so one ACTIVATE covers 1536 pT columns
    across stripe boundaries: 48 ACTIVATEs total (exp bound 84us -> 75us).
  * Phase 2+3 emission is one continuous stream: scores heads 0..3 is the
    main stream; a ready-gated ordered filler queue (leftover phase-1
    projections, per-stripe-eager PV for every head, o_proj blocks gated on
    PV/transposes) is paced by column debt, eliminating the window seams.
  * PSUM: s_ps 2x[128,1536] for scores/exp + mm_ps 2x[128,512] shared by
    projections, PV pairs and o_proj (8 banks exactly).
  * pT triple buffering (2-buf pool + 1-buf pool opened after phase-1 SBUF
    is released) so head a+1 scores start while head a-1 PV drains.
  * Startup DMA: wv/wq moved to the sync ring behind hs block 0 (ahead of
    hs block 1) so the weight ring only carries wk+trig early; cos/sin are
    loaded in a [0:512] slab first; exp act-table is preloaded at t=0.
  * o_proj psum->SBUF copies alternate ScalarE/DVE; outT stores go on the
    scalar ring; final stores are per-128-column so the end drain is short.

Built on bacc.Bacc: TRN2 instructions can carry at most ONE semaphore wait;
Bacc.compile() legalizes multi-wait instructions.
"""

import sys

sys.path.insert(0, "/opt/trn_rl_repo")

import numpy as np
from bisect import bisect_right
from contextlib import ExitStack

B = 2
S = 2048
H = 2048
D = 128
NQ = 4          # q heads per core
NKVL = 2        # kv heads per core
HC = H // 128   # 16 h-chunks (contraction)
NB = 4          # hs^T column blocks of 512 for projections
BW = S // NB    # 512
ST = S // 128   # 16 s-tiles / k-chunks / q-tiles
SCALE = 1.0 / np.sqrt(D)

# stripe c of the exp'd transposed scores covers q in [128c, S); offsets of
# the stripes packed into one [128, PT_TOTAL] sbuf tile
STRIPE_LEN = [S - 128 * c for c in range(ST)]
STRIPE_OFF = np.concatenate([[0], np.cumsum(STRIPE_LEN)]).tolist()
PT_TOTAL = STRIPE_OFF[-1]  # 17408

CH = 1536       # exp chunk width (3 psum banks)

MM_DT = "float16"

_CACHE = {}


def _build_program():
    import concourse.tile as tile
    from concourse import bacc, mybir

    f32 = mybir.dt.float32
    fmm = getattr(mybir.dt, MM_DT)
    nc = bacc.Bacc()

    hsT_d = nc.declare_dram_parameter("hsT", [NB, 128, HC, BW], fmm, isOutput=False)
    wq_d = nc.declare_dram_parameter("wq", [128, HC, 128 * NQ], fmm, isOutput=False)
    wk_d = nc.declare_dram_parameter("wk", [128, HC, 128 * NKVL], fmm, isOutput=False)
    wv_d = nc.declare_dram_parameter("wv", [128, HC, 128 * NKVL], fmm, isOutput=False)
    wo_d = nc.declare_dram_parameter("wo", [128, NQ, H], fmm, isOutput=False)
    cos_d = nc.declare_dram_parameter("cosf", [128, S], fmm, isOutput=False)
    sin_d = nc.declare_dram_parameter("sins", [128, S], fmm, isOutput=False)
    mask_d = nc.declare_dram_parameter("mask", [128, 128], fmm, isOutput=False)
    outT_d = nc.declare_dram_parameter("outT", [H, S], fmm, isOutput=True)

    with tile.TileContext(nc) as tc, ExitStack() as top:
        glob = top.enter_context(tc.tile_pool(name="glob", bufs=1))
        qrot = glob.tile([128, NQ, S], fmm)      # q^T, rope'd, per head
        krot = glob.tile([128, NKVL, S], fmm)    # k^T, rope'd, per kv head
        vaug = glob.tile([128, NKVL, ST, 132], fmm)  # v chunks + ones col @128
        attnT = glob.tile([128, NQ, ST, 128], fmm)  # attention out, transposed
        mask_sb = glob.tile([128, 128], fmm)
        warm = glob.tile([128, 1], f32)

        nc.vector.memset(vaug[:, :, :, 128:129], 1.0)
        nc.vector.memset(warm, 0.0)

        # ---------------- pools (stack allocator: long-lived first) --------
        ph2 = ExitStack()
        ptp = ph2.enter_context(tc.tile_pool(name="p2pt", bufs=2))
        stg = ph2.enter_context(tc.tile_pool(name="p2stg", bufs=6))
        smal = ph2.enter_context(tc.tile_pool(name="p2small", bufs=4))
        s_ps = ph2.enter_context(tc.tile_pool(name="p2sps", bufs=2, space="PSUM"))
        mm_ps = ph2.enter_context(tc.tile_pool(name="p2mm", bufs=2, space="PSUM"))

        ph1 = ExitStack()
        consts = ph1.enter_context(tc.tile_pool(name="p1const", bufs=1))
        hsp = ph1.enter_context(tc.tile_pool(name="p1hs", bufs=2))
        ropep = ph1.enter_context(tc.tile_pool(name="p1rope", bufs=3))

        # ---------------- phase 1 emission helpers ----------------
        def hs_load(nb, split_first=False):
            t = hsp.tile([128, HC, BW], fmm, name=f"hs_{nb}", tag="hs")
            if split_first:
                # block 0 in three slabs: chunk 0 alone so the very first
                # matmul starts immediately, then two large slabs that land
                # on early-starting DMA engines
                nc.sync.dma_start(out=t[:, 0:1, :], in_=hsT_d[nb, :, 0:1, :])
                nc.sync.dma_start(out=t[:, 1:8, :], in_=hsT_d[nb, :, 1:8, :])
                nc.sync.dma_start(out=t[:, 8:16, :], in_=hsT_d[nb, :, 8:16, :])
                return t
            nc.sync.dma_start(out=t[:, 0:4, :], in_=hsT_d[nb, :, 0:4, :])
            for c4 in range(4, HC, 4):
                nc.sync.dma_start(out=t[:, c4 : c4 + 4, :], in_=hsT_d[nb, :, c4 : c4 + 4, :])
            return t

        # startup loads.  sync ring pairs hs0 chunks with wk chunks in the
        # exact k-gen consumption order; scalar ring carries trig + wv + wq.
        wq_sb = consts.tile([128, HC, 128 * NQ], fmm)
        wk_sb = consts.tile([128, HC, 128 * NKVL], fmm)
        wv_sb = consts.tile([128, HC, 128 * NKVL], fmm)
        cos_sb = consts.tile([128, S], fmm)
        sin_sb = consts.tile([128, S], fmm)
        hs_tiles = [None] * NB
        # hs0 is split across BOTH rings: chunks 0-7 lead the sync ring while
        # chunks 8-15 follow wk on the scalar ring, so the (hs, wk) chunk
        # pairs the first k-gens consume arrive at the two-ring aggregate rate
        hs0 = hsp.tile([128, HC, BW], fmm, name="hs_0", tag="hs")
        hs_tiles[0] = hs0
        nc.sync.dma_start(out=hs0[:, 0:1, :], in_=hsT_d[0, :, 0:1, :])
        nc.sync.dma_start(out=hs0[:, 1:4, :], in_=hsT_d[0, :, 1:4, :])
        nc.sync.dma_start(out=hs0[:, 4:8, :], in_=hsT_d[0, :, 4:8, :])
        nc.scalar.dma_start(out=wk_sb[:, 0:1, :], in_=wk_d[:, 0:1, :])
        nc.scalar.dma_start(out=wk_sb[:, 1:8, :], in_=wk_d[:, 1:8, :])
        nc.scalar.dma_start(out=wk_sb[:, 8:16, :], in_=wk_d[:, 8:16, :])
        nc.scalar.dma_start(out=hs0[:, 8:12, :], in_=hsT_d[0, :, 8:12, :])
        nc.scalar.dma_start(out=hs0[:, 12:16, :], in_=hsT_d[0, :, 12:16, :])
        nc.scalar.dma_start(out=cos_sb[:, 0:512], in_=cos_d[:, 0:512])
        nc.scalar.dma_start(out=sin_sb[:, 0:512], in_=sin_d[:, 0:512])
        # preload the exp act-table (~2.7us) while the DMAs stream; placed
        # after the early scalar-ring issues so it doesn't delay them
        nc.scalar.activation(warm, warm, mybir.ActivationFunctionType.Exp)
        for c4 in range(0, HC, 4):
            nc.sync.dma_start(out=wv_sb[:, c4 : c4 + 4, :], in_=wv_d[:, c4 : c4 + 4, :])
        for c4 in range(0, HC, 4):
            nc.sync.dma_start(out=wq_sb[:, c4 : c4 + 4, :], in_=wq_d[:, c4 : c4 + 4, :])
        hs_tiles[1] = hs_load(1)
        nc.scalar.dma_start(out=cos_sb[:, 512:S], in_=cos_d[:, 512:S])
        nc.scalar.dma_start(out=sin_sb[:, 512:S], in_=sin_d[:, 512:S])
        nc.scalar.dma_start(out=mask_sb, in_=mask_d[:, :])

        def qk_tile_gen(nb, mt):
            """mt 0..3 = q heads, 4..5 = k heads. Yields cols after each mm."""
            n0 = nb * BW
            hs_t = hs_tiles[nb]
            ps = mm_ps.tile([128, BW], f32, tag="mmps")
            if mt < NQ:
                w_sb, mo = wq_sb, mt
            else:
                w_sb, mo = wk_sb, mt - NQ
            for c in range(HC):
                nc.tensor.matmul(
                    ps,
                    w_sb[:, c, 128 * mo : 128 * mo + 128],
                    hs_t[:, c, :],
                    start=(c == 0),
                    stop=(c == HC - 1),
                )
                yield BW
            if mt < NQ:
                dest = qrot[:, mt, n0 : n0 + BW]
            else:
                dest = krot[:, mt - NQ, n0 : n0 + BW]
            # rope: dest = ps * cos + swap_halves(ps) * (+/-)sin
            t_t = ropep.tile([128, BW], f32, tag="ropet")
            u_t = ropep.tile([128, BW], f32, tag="ropeu")
            nc.vector.tensor_mul(t_t, ps, cos_sb[:, n0 : n0 + BW])
            nc.vector.tensor_mul(u_t[0:64, :], ps[64:128, :], sin_sb[0:64, n0 : n0 + BW])
            nc.vector.tensor_mul(u_t[64:128, :], ps[0:64, :], sin_sb[64:128, n0 : n0 + BW])
            nc.vector.tensor_add(dest, t_t, u_t)

        def v_tile_gen(nb, st2):
            st = (BW // 128) * nb + st2
            hs_t = hs_tiles[nb]
            psw = mm_ps.tile([128, BW], f32, tag="mmps")
            ps = psw[:, 0 : 128 * NKVL]
            for c in range(HC):
                nc.tensor.matmul(
                    ps,
                    hs_t[:, c, 128 * st2 : 128 * st2 + 128],
                    wv_sb[:, c, :],
                    start=(c == 0),
                    stop=(c == HC - 1),
                )
                yield 128 * NKVL
            # single strided cast: [128, 2, 128] psum -> vaug[:, :, st, 0:128]
            nc.vector.tensor_copy(
                vaug[:, :, st, 0:128],
                ps.rearrange("p (kv d) -> p kv d", kv=NKVL),
            )

        def run(gen):
            for _ in gen:
                pass

        # ---------------- phase 2: scores main stream ----------------
        pT_tiles = [None] * NQ
        stripes_done = [0] * NQ   # fully exp'd + masked stripes per head
        pv_tiles = [0] * NQ       # PV output tiles emitted per head

        def scores_stream(a, pool):
            """Main-stream generator for head a: yields cols after each
            scores sub-matmul.  Exp chunks of CH pT columns, stripe packing,
            per-bank start/stop flags, masks + stripe bookkeeping."""
            kv = a // 2
            pT = pool.tile([128, PT_TOTAL], fmm, tag="pT")
            pT_tiles[a] = pT
            pos = 0
            masked = 0  # stripes masked so far
            while pos < PT_TOTAL:
                clen = min(CH, PT_TOTAL - pos)
                ps = s_ps.tile([128, CH], f32, tag="sps")
                seg = pos
                while seg < pos + clen:
                    c = bisect_right(STRIPE_OFF, seg) - 1
                    send = STRIPE_OFF[c + 1]
                    boff = seg - pos
                    bank_end = pos + (boff // 512 + 1) * 512
                    end = min(send, bank_end)
                    w = end - seg
                    qcol = 128 * c + (seg - STRIPE_OFF[c])
                    first_in_bank = (boff % 512) == 0
                    last_in_bank = end == bank_end
                    nc.tensor.matmul(
                        ps[:, boff : boff + w],
                        krot[:, kv, 128 * c : 128 * c + 128],
                        qrot[:, a, qcol : qcol + w],
                        start=first_in_bank,
                        stop=last_in_bank,
                        skip_group_check=not (first_in_bank and last_in_bank),
                    )
                    yield w
                    seg = end
                nc.scalar.activation(
                    pT[:, pos : pos + clen],
                    ps[:, 0:clen],
                    mybir.ActivationFunctionType.Exp,
                    scale=float(SCALE),
                )
                pos += clen
                # masks for newly covered diagonal blocks
                while masked < ST and STRIPE_OFF[masked] + 128 <= pos:
                    off = STRIPE_OFF[masked]
                    nc.vector.tensor_mul(
                        pT[:, off : off + 128], pT[:, off : off + 128], mask_sb
                    )
                    masked += 1
                # stripe completion (exp coverage + mask emitted)
                nd = bisect_right(STRIPE_OFF, pos) - 1
                stripes_done[a] = min(nd, masked)

        # ---------------- PV ----------------
        pv_stage = [None] * NQ

        def pv_pair_gen(a, t0):
            """PV + normalize for tiles t0, t0+1 sharing one PSUM bank:
            chain t0 at cols [0:129], t0+1 at [132:261].  The start=True
            matmul of chain t0 zeroes the whole 2KB bank, so chain t0+1
            accumulates with start=False throughout.  Two pairs share one
            [128,512] stage; the pair at t0%4==2 emits a single batched
            XBAR transpose covering the 4 tiles of o_proj block t0//4."""
            kv = a // 2
            pT = pT_tiles[a]
            t1 = t0 + 1
            po = mm_ps.tile([128, BW], f32, tag="mmps")
            for c in range(t1 + 1):
                if c <= t0:
                    lhsT = pT[
                        :,
                        STRIPE_OFF[c] + 128 * (t0 - c) : STRIPE_OFF[c] + 128 * (t0 - c) + 128,
                    ]
                    nc.tensor.matmul(
                        po[:, 0:129],
                        lhsT,
                        vaug[:, kv, c, 0:129],
                        start=(c == 0),
                        stop=(c == t0),
                        skip_group_check=True,
                    )
                    yield 258
                lhsT = pT[
                    :,
                    STRIPE_OFF[c] + 128 * (t1 - c) : STRIPE_OFF[c] + 128 * (t1 - c) + 128,
                ]
                nc.tensor.matmul(
                    po[:, 132:261],
                    lhsT,
                    vaug[:, kv, c, 0:129],
                    start=False,
                    stop=(c == t1),
                    skip_group_check=True,
                )
            yield 129
            if t0 % 4 == 0:
                pv_stage[a] = stg.tile([128, 512], fmm, name=f"stg_{a}_{t0}", tag="stage")
            stage = pv_stage[a]
            so = 256 * ((t0 % 4) // 2)
            for j, st2 in ((0, 0), (132, 1)):
                r = smal.tile([128, 1], f32, tag="recip")
                nc.vector.reciprocal(r, po[:, j + 128 : j + 129])
                nc.vector.tensor_scalar_mul(
                    stage[:, so + 128 * st2 : so + 128 * st2 + 128], po[:, j : j + 128], r
                )
            if t0 % 4 == 2:
                eng = nc.scalar if (scalar_free[0] and t0 % 8 == 6) else nc.sync
                eng.dma_start(
                    out=attnT[:, a, t0 - 2 : t0 + 2, :], in_=stage, transpose=True
                )
            pv_tiles[a] = t1 + 1

        # ---------------- o_proj ----------------
        outT_v = outT_d.rearrange("(m p) s -> p m s", p=128)
        o_copy_flip = [0]
        scalar_free = [False]  # True once all exps are emitted
        o_sps = {"tile": None, "k": 0}

        def o_mt_step(ns, mt, wo_sb, ostg_tiles):
            if mt == 0:
                ostg_tiles[ns] = ostg.tile(
                    [128, H // 128, BW], fmm, name=f"ostg_{ns}", tag="ostg"
                )
            ot = ostg_tiles[ns]
            if scalar_free[0] and pv_tiles[3] >= ST:
                # exps + pv done: rotate o_proj psum through all 8 banks
                # (freed scores banks + mm banks) so copy latency never
                # gates the matmuls
                k = o_sps["k"] % 4
                o_sps["k"] += 1
                if k == 3:
                    ps = mm_ps.tile([128, BW], f32, tag="mmps")
                else:
                    if k == 0:
                        o_sps["tile"] = s_ps.tile([128, CH], f32, name="osps", tag="sps")
                    ps = o_sps["tile"][:, 512 * k : 512 * k + 512]
            else:
                ps = mm_ps.tile([128, BW], f32, tag="mmps")
            for a in range(NQ):
                nc.tensor.matmul(
                    ps,
                    wo_sb[:, a, 128 * mt : 128 * mt + 128],
                    attnT[:, a, 4 * ns : 4 * ns + 4, :].rearrange("p t d -> p (t d)"),
                    start=(a == 0),
                    stop=(a == NQ - 1),
                )
            # ScalarE must stay exp-only until the last exp is emitted:
            # anything queued ahead of an exp head-of-line blocks it
            if scalar_free[0] and o_copy_flip[0] == 0:
                nc.scalar.copy(ot[:, mt, :], ps)
            else:
                nc.vector.tensor_copy(ot[:, mt, :], ps)
            o_copy_flip[0] ^= 1
            fine = ns == S // BW - 1 and mt >= 12
            step = 1 if fine else 2
            if (mt + 1) % step == 0:
                # the tail stores ride the scalar HW-DGE ring (ScalarE is
                # idle then and HW-DGE drains faster than gpsimd SW-DGE)
                eng = nc.scalar if fine else nc.gpsimd
                eng.dma_start(
                    out=outT_v[:, mt - step + 1 : mt + 1, BW * ns : BW * ns + BW],
                    in_=ot[:, mt - step + 1 : mt + 1, :],
                )
            return NQ * BW

        # ---------------- filler queue ----------------
        class GenF:
            """Wraps a generator yielding cost units; always ready."""
            def __init__(self, gen):
                self.gen = gen
                self.fin = False
            def ready(self):
                return True
            def emit(self):
                try:
                    return next(self.gen)
                except StopIteration:
                    self.fin = True
                    return None
            def done(self):
                return self.fin

        class PVF:
            def __init__(self, a):
                self.a = a
                self.t0 = 0
                self.cur = None
            def ready(self):
                if self.cur is not None:
                    return True
                return stripes_done[self.a] >= self.t0 + 2
            def emit(self):
                if self.cur is None:
                    self.cur = pv_pair_gen(self.a, self.t0)
                try:
                    return next(self.cur)
                except StopIteration:
                    self.cur = None
                    self.t0 += 2
                    if self.t0 >= ST:
                        return None
                    if stripes_done[self.a] >= self.t0 + 2:
                        self.cur = pv_pair_gen(self.a, self.t0)
                        return next(self.cur)
                    return None
            def done(self):
                return self.t0 >= ST and self.cur is None

        O_LAG = 4

        class OF:
            def __init__(self, ns, wo_sb, ostg_tiles):
                self.ns = ns
                self.mt = 0
                self.wo_sb = wo_sb
                self.ostg_tiles = ostg_tiles
            def ready(self):
                need = 4 * self.ns + 4
                for a in range(3):
                    if pv_tiles[a] < need:
                        return False
                return pv_tiles[3] >= min(ST, need + O_LAG)
            def emit(self):
                cost = o_mt_step(self.ns, self.mt, self.wo_sb, self.ostg_tiles)
                self.mt += 1
                return cost
            def done(self):
                return self.mt >= H // 128

        filler_q = []

        def pull(force=False):
            """Emit one unit from the first ready filler; returns cost or
            None.  force: emit the first unfinished filler even if gated."""
            for f in filler_q:
                if not f.done() and f.ready():
                    c = f.emit()
                    if c is not None:
                        return c
            if force:
                for f in filler_q:
                    if not f.done():
                        c = f.emit()
                        if c is not None:
                            return c
            return None

        def drive(main_gen, ratio):
            debt = 2000.0
            for w in main_gen:
                debt += w * ratio
                while debt > 0:
                    c = pull()
                    if c is None:
                        break
                    debt -= c

        def drain_fillers(sel=None):
            while True:
                got = False
                for f in filler_q:
                    if sel is not None and f not in sel:
                        continue
                    if not f.done():
                        c = f.emit()
                        got = True
                        break
                if not got:
                    break

        # ---------------- phase 1 blocks 0..2 ----------------
        for nb in range(3):
            if nb >= 1:
                hs_tiles[nb + 1] = hs_load(nb + 1)
            run(qk_tile_gen(nb, 4))
            run(qk_tile_gen(nb, 5))
            for st2 in range(BW // 128):
                run(v_tile_gen(nb, st2))
            for mt in range(NQ):
                run(qk_tile_gen(nb, mt))

        # ---------------- block 3 head/k parts, then streamed ph2 ----------
        run(qk_tile_gen(3, 4))
        run(qk_tile_gen(3, 5))
        run(qk_tile_gen(3, 0))

        RATIO = 1.5

        # window 0: scores head 0; fillers = remaining projections
        f_q1 = GenF(qk_tile_gen(3, 1))
        f_q2 = GenF(qk_tile_gen(3, 2))
        f_v30 = GenF(v_tile_gen(3, 0))
        f_v31 = GenF(v_tile_gen(3, 1))
        f_q3 = GenF(qk_tile_gen(3, 3))
        f_v32 = GenF(v_tile_gen(3, 2))
        f_v33 = GenF(v_tile_gen(3, 3))
        ph1_tail = [f_q1, f_q2, f_v30, f_v31, f_q3, f_v32, f_v33]
        filler_q.extend(ph1_tail)
        filler_q.append(PVF(0))

        drive(scores_stream(0, ptp), RATIO)
        filler_q.append(PVF(1))
        drive(scores_stream(1, ptp), RATIO)

        # all phase-1 consumers must be emitted before releasing its SBUF
        drain_fillers(sel=ph1_tail)
        ph1.close()
        ptp_x = ExitStack()
        ptpx = ptp_x.enter_context(tc.tile_pool(name="p2ptx", bufs=1))
        wop = ptp_x.enter_context(tc.tile_pool(name="p3wo", bufs=1))
        ostg_stk = ExitStack()
        ostg = ostg_stk.enter_context(tc.tile_pool(name="p3stg", bufs=2))
        wo_sb = wop.tile([128, NQ, H], fmm)
        # gpsimd SW-DGE ring: keeps the ScalarE and SP queues free for
        # exps / transposes (in-order queues head-of-line block otherwise)
        for a in range(NQ):
            nc.gpsimd.dma_start(out=wo_sb[:, a, :], in_=wo_d[:, a, :])

        filler_q.append(PVF(2))
        drive(scores_stream(2, ptpx), RATIO)

        # head 3 reuses the first main pT buffer: pv0 must be fully emitted
        drain_fillers(sel=[f for f in filler_q if isinstance(f, PVF) and f.a == 0])
        ostg_tiles = [None] * (S // BW)
        filler_q.append(PVF(3))
        for ns in range(S // BW):
            filler_q.append(OF(ns, wo_sb, ostg_tiles))
        drive(scores_stream(3, ptp), RATIO)
        scalar_free[0] = True

        # drain: pv leftovers + o_proj, ready-gated then forced
        while True:
            c = pull()
            if c is None:
                c = pull(force=True)
                if c is None:
                    break

        ostg_stk.close()
        ptp_x.close()
        ph2.close()

    nc.finalize()
    return nc


def _rope_tables():
    inv_freq = 1.0 / (10000.0 ** (np.arange(0, D, 2, dtype=np.float32) / D))
    t = np.arange(S, dtype=np.float32)[:, None]
    freqs = t * inv_freq[None, :]          # [S, 64]
    cos = np.cos(freqs).astype(np.float32)  # [S, 64]
    sin = np.sin(freqs).astype(np.float32)
    mdt = np.dtype(MM_DT)
    cosf = np.concatenate([cos, cos], axis=1).T.astype(mdt)    # [128, S]
    sins = np.concatenate([-sin, sin], axis=1).T.astype(mdt)   # [128, S]
    return np.ascontiguousarray(cosf), np.ascontiguousarray(sins)


def _prep_in_maps(hidden_states, Wq, Wk, Wv, Wo):
    mdt = np.dtype(MM_DT)
    cosf, sins = _rope_tables()
    mask = np.triu(np.ones((128, 128), dtype=mdt))  # [j, q]: 1 if j <= q

    hsT_blocks = []
    for b in range(B):
        hsT = hidden_states[b].T  # [H, S]
        blk = np.ascontiguousarray(
            hsT.reshape(HC, 128, NB, BW).transpose(2, 1, 0, 3).astype(mdt)
        )  # [NB, 128, HC, BW]
        hsT_blocks.append(blk)

    in_maps = []
    for i in range(8):
        b, g = i // 4, i % 4
        wq = np.ascontiguousarray(
            Wq[512 * g : 512 * (g + 1), :].reshape(512, HC, 128).transpose(2, 1, 0).astype(mdt)
        )
        wk = np.ascontiguousarray(
            Wk[256 * g : 256 * (g + 1), :].reshape(256, HC, 128).transpose(2, 1, 0).astype(mdt)
        )
        wv = np.ascontiguousarray(
            Wv[256 * g : 256 * (g + 1), :].reshape(256, HC, 128).transpose(2, 1, 0).astype(mdt)
        )
        wo = np.ascontiguousarray(
            Wo[:, 512 * g : 512 * (g + 1)].reshape(H, NQ, 128).transpose(2, 1, 0).astype(mdt)
        )
        in_maps.append(
            {
                "hsT": hsT_blocks[b],
                "wq": wq,
                "wk": wk,
                "wv": wv,
                "wo": wo,
                "cosf": cosf,
                "sins": sins,
                "mask": mask,
            }
        )
    return in_maps


def _run(in_maps, **kwargs):
    from concourse.bass_utils import run_bass_kernel_spmd

    if "prog" not in _CACHE:
        _CACHE["prog"] = _build_program()
    nc = _CACHE["prog"]
    return run_bass_kernel_spmd(nc, in_maps, core_ids=list(range(8)), **kwargs)


def _gather(results):
    out = np.empty((B, S, H), dtype=np.float32)
    for b in range(B):
        acc = results[4 * b + 0]["outT"].astype(np.float32)
        for g in range(1, 4):
            acc += results[4 * b + g]["outT"].astype(np.float32)
        out[b] = acc.T
    return out


def kernel(hidden_states, Wq, Wk, Wv, Wo):
    hidden_states = np.asarray(hidden_states, dtype=np.float32)
    Wq = np.asarray(Wq, dtype=np.float32)
    Wk = np.asarray(Wk, dtype=np.float32)
    Wv = np.asarray(Wv, dtype=np.float32)
    Wo = np.asarray(Wo, dtype=np.float32)
    in_maps = _prep_in_maps(hidden_states, Wq, Wk, Wv, Wo)
    res = _run(in_maps)
    return _gather(res.results)


# revision 33
# speedup vs baseline: 1.1836x; 1.1836x over previous
"""Trainium2 Bass kernel for GQA attention block (nn_Attention_20272245637793).

Reference computation (B=2, S=2048, H=2048, 16 q heads / 8 kv heads, D=128):
    q = hs @ Wq.T ; k = hs @ Wk.T ; v = hs @ Wv.T
    rope(q), rope(k); causal softmax(q k^T / sqrt(D)) @ v ; out @ Wo.T

Sharding (8 cores): core i = (b, g) with b = i // 4 (data-parallel over
batch), g = i % 4 (tensor-parallel over kv-head groups; kv heads {2g, 2g+1},
q heads {4g..4g+3}).  Each core computes 1/8 of every GEMM and a partial
o_proj over its 512 head-dims; the host sums the 4 partials per batch.

v3 changes over v2 (trace-driven; v2 = 280.5us, PE busy 84%):
  * Phase 2 was ScalarE-exp-bound: 96 ACTIVATEs x (cols*0.833ns + ~274ns
    fixed).  Scores psum tiles are now [128,1536] (3 banks) and the exp'd
    stripes are packed contiguously, so one ACTIVATE covers 1536 pT columns
    across stripe boundaries: 48 ACTIVATEs total (exp bound 84us -> 75us).
  * Phase 2+3 emission is one continuous stream: scores heads 0..3 is the
    main stream; a ready-gated ordered filler queue (leftover phase-1
    projections, per-stripe-eager PV for every head, o_proj blocks gated on
    PV/transposes) is paced by column debt, eliminating the window seams.
  * PSUM: s_ps 2x[128,1536] for scores/exp + mm_ps 2x[128,512] shared by
    projections, PV pairs and o_proj (8 banks exactly).
  * pT triple buffering (2-buf pool + 1-buf pool opened after phase-1 SBUF
    is released) so head a+1 scores start while head a-1 PV drains.
  * Startup DMA: wv/wq moved to the sync ring behind hs block 0 (ahead of
    hs block 1) so the weight ring only carries wk+trig early; cos/sin are
    loaded in a [0:512] slab first; exp act-table is preloaded at t=0.
  * o_proj psum->SBUF copies alternate ScalarE/DVE; outT stores go on the
    scalar ring; final stores are per-128-column so the end drain is short.

Built on bacc.Bacc: TRN2 instructions can carry at most ONE semaphore wait;
Bacc.compile() legalizes multi-wait instructions.
"""

import sys

sys.path.insert(0, "/opt/trn_rl_repo")

import numpy as np
from bisect import bisect_right
from contextlib import ExitStack

B = 2
S = 2048
H = 2048
D = 128
NQ = 4          # q heads per core
NKVL = 2        # kv heads per core
HC = H // 128   # 16 h-chunks (contraction)
NB = 4          # hs^T column blocks of 512 for projections
BW = S // NB    # 512
ST = S // 128   # 16 s-tiles / k-chunks / q-tiles
SCALE = 1.0 / np.sqrt(D)

# stripe c of the exp'd transposed scores covers q in [128c, S); offsets of
# the stripes packed into one [128, PT_TOTAL] sbuf tile
STRIPE_LEN = [S - 128 * c for c in range(ST)]
STRIPE_OFF = np.concatenate([[0], np.cumsum(STRIPE_LEN)]).tolist()
PT_TOTAL = STRIPE_OFF[-1]  # 17408

CH = 1536       # exp chunk width (3 psum banks)

MM_DT = "float16"

_CACHE = {}


def _build_program():
    import concourse.tile as tile
    from concourse import bacc, mybir

    f32 = mybir.dt.float32
    fmm = getattr(mybir.dt, MM_DT)
    nc = bacc.Bacc()

    hsT_d = nc.declare_dram_parameter("hsT", [NB, 128, HC, BW], fmm, isOutput=False)
    wq_d = nc.declare_dram_parameter("wq", [128, HC, 128 * NQ], fmm, isOutput=False)
    wk_d = nc.declare_dram_parameter("wk", [128, HC, 128 * NKVL], fmm, isOutput=False)
    wv_d = nc.declare_dram_parameter("wv", [128, HC, 128 * NKVL], fmm, isOutput=False)
    wo_d = nc.declare_dram_parameter("wo", [128, NQ, H], fmm, isOutput=False)
    cos_d = nc.declare_dram_parameter("cosf", [128, S], fmm, isOutput=False)
    sin_d = nc.declare_dram_parameter("sins", [128, S], fmm, isOutput=False)
    mask_d = nc.declare_dram_parameter("mask", [128, 128], fmm, isOutput=False)
    outT_d = nc.declare_dram_parameter("outT", [H, S], fmm, isOutput=True)

    with tile.TileContext(nc) as tc, ExitStack() as top:
        glob = top.enter_context(tc.tile_pool(name="glob", bufs=1))
        qrot = glob.tile([128, NQ, S], fmm)      # q^T, rope'd, per head
        krot = glob.tile([128, NKVL, S], fmm)    # k^T, rope'd, per kv head
        vaug = glob.tile([128, NKVL, ST, 132], fmm)  # v chunks + ones col @128
        attnT = glob.tile([128, NQ, ST, 128], fmm)  # attention out, transposed
        mask_sb = glob.tile([128, 128], fmm)
        warm = glob.tile([128, 1], f32)
        pewarm = glob.tile([128, 128], fmm)

        nc.vector.memset(vaug[:, :, :, 128:129], 1.0)
        nc.vector.memset(warm, 0.0)
        nc.vector.memset(pewarm, 0.0)

        # ---------------- pools (stack allocator: long-lived first) --------
        ph2 = ExitStack()
        ptp = ph2.enter_context(tc.tile_pool(name="p2pt", bufs=2))
        stg = ph2.enter_context(tc.tile_pool(name="p2stg", bufs=6))
        smal = ph2.enter_context(tc.tile_pool(name="p2small", bufs=4))
        s_ps = ph2.enter_context(tc.tile_pool(name="p2sps", bufs=2, space="PSUM"))
        mm_ps = ph2.enter_context(tc.tile_pool(name="p2mm", bufs=2, space="PSUM"))

        ph1 = ExitStack()
        consts = ph1.enter_context(tc.tile_pool(name="p1const", bufs=1))
        hsp = ph1.enter_context(tc.tile_pool(name="p1hs", bufs=2))
        ropep = ph1.enter_context(tc.tile_pool(name="p1rope", bufs=3))

        # ---------------- phase 1 emission helpers ----------------
        def hs_load(nb, split_first=False):
            t = hsp.tile([128, HC, BW], fmm, name=f"hs_{nb}", tag="hs")
            if split_first:
                # block 0 in three slabs: chunk 0 alone so the very first
                # matmul starts immediately, then two large slabs that land
                # on early-starting DMA engines
                nc.sync.dma_start(out=t[:, 0:1, :], in_=hsT_d[nb, :, 0:1, :])
                nc.sync.dma_start(out=t[:, 1:8, :], in_=hsT_d[nb, :, 1:8, :])
                nc.sync.dma_start(out=t[:, 8:16, :], in_=hsT_d[nb, :, 8:16, :])
                return t
            nc.sync.dma_start(out=t[:, 0:4, :], in_=hsT_d[nb, :, 0:4, :])
            for c4 in range(4, HC, 4):
                nc.sync.dma_start(out=t[:, c4 : c4 + 4, :], in_=hsT_d[nb, :, c4 : c4 + 4, :])
            return t

        # startup loads.  sync ring pairs hs0 chunks with wk chunks in the
        # exact k-gen consumption order; scalar ring carries trig + wv + wq.
        wq_sb = consts.tile([128, HC, 128 * NQ], fmm)
        wk_sb = consts.tile([128, HC, 128 * NKVL], fmm)
        wv_sb = consts.tile([128, HC, 128 * NKVL], fmm)
        cos_sb = consts.tile([128, S], fmm)
        sin_sb = consts.tile([128, S], fmm)
        hs_tiles = [None] * NB
        hs_tiles[0] = hs_load(0, split_first=True)
        nc.scalar.dma_start(out=wk_sb[:, 0:1, :], in_=wk_d[:, 0:1, :])
        nc.scalar.dma_start(out=wk_sb[:, 1:8, :], in_=wk_d[:, 1:8, :])
        nc.scalar.dma_start(out=wk_sb[:, 8:16, :], in_=wk_d[:, 8:16, :])
        nc.scalar.dma_start(out=cos_sb[:, 0:512], in_=cos_d[:, 0:512])
        nc.scalar.dma_start(out=sin_sb[:, 0:512], in_=sin_d[:, 0:512])
        # preload the exp act-table (~2.7us) while the DMAs stream; placed
        # after the early scalar-ring issues so it doesn't delay them
        nc.scalar.activation(warm, warm, mybir.ActivationFunctionType.Exp)
        for c4 in range(0, HC, 4):
            nc.sync.dma_start(out=wv_sb[:, c4 : c4 + 4, :], in_=wv_d[:, c4 : c4 + 4, :])
        for c4 in range(0, HC, 4):
            nc.sync.dma_start(out=wq_sb[:, c4 : c4 + 4, :], in_=wq_d[:, c4 : c4 + 4, :])
        hs_tiles[1] = hs_load(1)
        nc.scalar.dma_start(out=cos_sb[:, 512:S], in_=cos_d[:, 512:S])
        nc.scalar.dma_start(out=sin_sb[:, 512:S], in_=sin_d[:, 512:S])
        nc.scalar.dma_start(out=mask_sb, in_=mask_d[:, :])

        # PE clock warm-up: the tensor engine idles ~3.5us waiting for the
        # first DMA data and then ramps its clock over ~3us of execution.
        # Grind small dummy matmuls on memset scratch during the wait so the
        # real projections start at full clock.  128-col grains keep the
        # overshoot into real work negligible.
        wps = mm_ps.tile([128, 128], f32, name="warmps", tag="mmps")
        for _ in range(32):
            nc.tensor.matmul(wps, pewarm, pewarm, start=True, stop=True)

        def qk_tile_gen(nb, mt):
            """mt 0..3 = q heads, 4..5 = k heads. Yields cols after each mm."""
            n0 = nb * BW
            hs_t = hs_tiles[nb]
            ps = mm_ps.tile([128, BW], f32, tag="mmps")
            if mt < NQ:
                w_sb, mo = wq_sb, mt
            else:
                w_sb, mo = wk_sb, mt - NQ
            for c in range(HC):
                nc.tensor.matmul(
                    ps,
                    w_sb[:, c, 128 * mo : 128 * mo + 128],
                    hs_t[:, c, :],
                    start=(c == 0),
                    stop=(c == HC - 1),
                )
                yield BW
            if mt < NQ:
                dest = qrot[:, mt, n0 : n0 + BW]
            else:
                dest = krot[:, mt - NQ, n0 : n0 + BW]
            # rope: dest = ps * cos + swap_halves(ps) * (+/-)sin
            t_t = ropep.tile([128, BW], f32, tag="ropet")
            u_t = ropep.tile([128, BW], f32, tag="ropeu")
            nc.vector.tensor_mul(t_t, ps, cos_sb[:, n0 : n0 + BW])
            nc.vector.tensor_mul(u_t[0:64, :], ps[64:128, :], sin_sb[0:64, n0 : n0 + BW])
            nc.vector.tensor_mul(u_t[64:128, :], ps[0:64, :], sin_sb[64:128, n0 : n0 + BW])
            nc.vector.tensor_add(dest, t_t, u_t)

        def v_tile_gen(nb, st2):
            st = (BW // 128) * nb + st2
            hs_t = hs_tiles[nb]
            psw = mm_ps.tile([128, BW], f32, tag="mmps")
            ps = psw[:, 0 : 128 * NKVL]
            for c in range(HC):
                nc.tensor.matmul(
                    ps,
                    hs_t[:, c, 128 * st2 : 128 * st2 + 128],
                    wv_sb[:, c, :],
                    start=(c == 0),
                    stop=(c == HC - 1),
                )
                yield 128 * NKVL
            # single strided cast: [128, 2, 128] psum -> vaug[:, :, st, 0:128]
            nc.vector.tensor_copy(
                vaug[:, :, st, 0:128],
                ps.rearrange("p (kv d) -> p kv d", kv=NKVL),
            )

        def run(gen):
            for _ in gen:
                pass

        # ---------------- phase 2: scores main stream ----------------
        pT_tiles = [None] * NQ
        stripes_done = [0] * NQ   # fully exp'd + masked stripes per head
        pv_tiles = [0] * NQ       # PV output tiles emitted per head

        def scores_stream(a, pool):
            """Main-stream generator for head a: yields cols after each
            scores sub-matmul.  Exp chunks of CH pT columns, stripe packing,
            per-bank start/stop flags, masks + stripe bookkeeping."""
            kv = a // 2
            pT = pool.tile([128, PT_TOTAL], fmm, tag="pT")
            pT_tiles[a] = pT
            pos = 0
            masked = 0  # stripes masked so far
            while pos < PT_TOTAL:
                clen = min(CH, PT_TOTAL - pos)
                ps = s_ps.tile([128, CH], f32, tag="sps")
                seg = pos
                while seg < pos + clen:
                    c = bisect_right(STRIPE_OFF, seg) - 1
                    send = STRIPE_OFF[c + 1]
                    boff = seg - pos
                    bank_end = pos + (boff // 512 + 1) * 512
                    end = min(send, bank_end)
                    w = end - seg
                    qcol = 128 * c + (seg - STRIPE_OFF[c])
                    first_in_bank = (boff % 512) == 0
                    last_in_bank = end == bank_end
                    nc.tensor.matmul(
                        ps[:, boff : boff + w],
                        krot[:, kv, 128 * c : 128 * c + 128],
                        qrot[:, a, qcol : qcol + w],
                        start=first_in_bank,
                        stop=last_in_bank,
                        skip_group_check=not (first_in_bank and last_in_bank),
                    )
                    yield w
                    seg = end
                nc.scalar.activation(
                    pT[:, pos : pos + clen],
                    ps[:, 0:clen],
                    mybir.ActivationFunctionType.Exp,
                    scale=float(SCALE),
                )
                pos += clen
                # masks for newly covered diagonal blocks
                while masked < ST and STRIPE_OFF[masked] + 128 <= pos:
                    off = STRIPE_OFF[masked]
                    nc.vector.tensor_mul(
                        pT[:, off : off + 128], pT[:, off : off + 128], mask_sb
                    )
                    masked += 1
                # stripe completion (exp coverage + mask emitted)
                nd = bisect_right(STRIPE_OFF, pos) - 1
                stripes_done[a] = min(nd, masked)

        # ---------------- PV ----------------
        pv_stage = [None] * NQ

        def pv_pair_gen(a, t0):
            """PV + normalize for tiles t0, t0+1 sharing one PSUM bank:
            chain t0 at cols [0:129], t0+1 at [132:261].  The start=True
            matmul of chain t0 zeroes the whole 2KB bank, so chain t0+1
            accumulates with start=False throughout.  Two pairs share one
            [128,512] stage; the pair at t0%4==2 emits a single batched
            XBAR transpose covering the 4 tiles of o_proj block t0//4."""
            kv = a // 2
            pT = pT_tiles[a]
            t1 = t0 + 1
            po = mm_ps.tile([128, BW], f32, tag="mmps")
            for c in range(t1 + 1):
                if c <= t0:
                    lhsT = pT[
                        :,
                        STRIPE_OFF[c] + 128 * (t0 - c) : STRIPE_OFF[c] + 128 * (t0 - c) + 128,
                    ]
                    nc.tensor.matmul(
                        po[:, 0:129],
                        lhsT,
                        vaug[:, kv, c, 0:129],
                        start=(c == 0),
                        stop=(c == t0),
                        skip_group_check=True,
                    )
                    yield 258
                lhsT = pT[
                    :,
                    STRIPE_OFF[c] + 128 * (t1 - c) : STRIPE_OFF[c] + 128 * (t1 - c) + 128,
                ]
                nc.tensor.matmul(
                    po[:, 132:261],
                    lhsT,
                    vaug[:, kv, c, 0:129],
                    start=False,
                    stop=(c == t1),
                    skip_group_check=True,
                )
            yield 129
            if t0 % 4 == 0:
                pv_stage[a] = stg.tile([128, 512], fmm, name=f"stg_{a}_{t0}", tag="stage")
            stage = pv_stage[a]
            so = 256 * ((t0 % 4) // 2)
            for j, st2 in ((0, 0), (132, 1)):
                r = smal.tile([128, 1], f32, tag="recip")
                nc.vector.reciprocal(r, po[:, j + 128 : j + 129])
                nc.vector.tensor_scalar_mul(
                    stage[:, so + 128 * st2 : so + 128 * st2 + 128], po[:, j : j + 128], r
                )
            if t0 % 4 == 2:
                eng = nc.scalar if (scalar_free[0] and t0 % 8 == 6) else nc.sync
                eng.dma_start(
                    out=attnT[:, a, t0 - 2 : t0 + 2, :], in_=stage, transpose=True
                )
            pv_tiles[a] = t1 + 1

        # ---------------- o_proj ----------------
        outT_v = outT_d.rearrange("(m p) s -> p m s", p=128)
        o_copy_flip = [0]
        scalar_free = [False]  # True once all exps are emitted
        o_sps = {"tile": None, "k": 0}

        def o_mt_step(ns, mt, wo_sb, ostg_tiles):
            if mt == 0:
                ostg_tiles[ns] = ostg.tile(
                    [128, H // 128, BW], fmm, name=f"ostg_{ns}", tag="ostg"
                )
            ot = ostg_tiles[ns]
            if scalar_free[0] and pv_tiles[3] >= ST:
                # exps + pv done: rotate o_proj psum through all 8 banks
                # (freed scores banks + mm banks) so copy latency never
                # gates the matmuls
                k = o_sps["k"] % 4
                o_sps["k"] += 1
                if k == 3:
                    ps = mm_ps.tile([128, BW], f32, tag="mmps")
                else:
                    if k == 0:
                        o_sps["tile"] = s_ps.tile([128, CH], f32, name="osps", tag="sps")
                    ps = o_sps["tile"][:, 512 * k : 512 * k + 512]
            else:
                ps = mm_ps.tile([128, BW], f32, tag="mmps")
            for a in range(NQ):
                nc.tensor.matmul(
                    ps,
                    wo_sb[:, a, 128 * mt : 128 * mt + 128],
                    attnT[:, a, 4 * ns : 4 * ns + 4, :].rearrange("p t d -> p (t d)"),
                    start=(a == 0),
                    stop=(a == NQ - 1),
                )
            # ScalarE must stay exp-only until the last exp is emitted:
            # anything queued ahead of an exp head-of-line blocks it
            if scalar_free[0] and o_copy_flip[0] == 0:
                nc.scalar.copy(ot[:, mt, :], ps)
            else:
                nc.vector.tensor_copy(ot[:, mt, :], ps)
            o_copy_flip[0] ^= 1
            fine = ns == S // BW - 1 and mt >= 12
            step = 1 if fine else 2
            if (mt + 1) % step == 0:
                # the tail stores ride the scalar HW-DGE ring (ScalarE is
                # idle then and HW-DGE drains faster than gpsimd SW-DGE)
                eng = nc.scalar if fine else nc.gpsimd
                eng.dma_start(
                    out=outT_v[:, mt - step + 1 : mt + 1, BW * ns : BW * ns + BW],
                    in_=ot[:, mt - step + 1 : mt + 1, :],
                )
            return NQ * BW

        # ---------------- filler queue ----------------
        class GenF:
            """Wraps a generator yielding cost units; always ready."""
            def __init__(self, gen):
                self.gen = gen
                self.fin = False
            def ready(self):
                return True
            def emit(self):
                try:
                    return next(self.gen)
                except StopIteration:
                    self.fin = True
                    return None
            def done(self):
                return self.fin

        class PVF:
            def __init__(self, a):
                self.a = a
                self.t0 = 0
                self.cur = None
            def ready(self):
                if self.cur is not None:
                    return True
                return stripes_done[self.a] >= self.t0 + 2
            def emit(self):
                if self.cur is None:
                    self.cur = pv_pair_gen(self.a, self.t0)
                try:
                    return next(self.cur)
                except StopIteration:
                    self.cur = None
                    self.t0 += 2
                    if self.t0 >= ST:
                        return None
                    if stripes_done[self.a] >= self.t0 + 2:
                        self.cur = pv_pair_gen(self.a, self.t0)
                        return next(self.cur)
                    return None
            def done(self):
                return self.t0 >= ST and self.cur is None

        O_LAG = 4

        class OF:
            def __init__(self, ns, wo_sb, ostg_tiles):
                self.ns = ns
                self.mt = 0
                self.wo_sb = wo_sb
                self.ostg_tiles = ostg_tiles
            def ready(self):
                need = 4 * self.ns + 4
                for a in range(3):
                    if pv_tiles[a] < need:
                        return False
                return pv_tiles[3] >= min(ST, need + O_LAG)
            def emit(self):
                cost = o_mt_step(self.ns, self.mt, self.wo_sb, self.ostg_tiles)
                self.mt += 1
                return cost
            def done(self):
                return self.mt >= H // 128

        filler_q = []

        def pull(force=False):
            """Emit one unit from the first ready filler; returns cost or
            None.  force: emit the first unfinished filler even if gated."""
            for f in filler_q:
                if not f.done() and f.ready():
                    c = f.emit()
                    if c is not None:
                        return c
            if force:
                for f in filler_q:
                    if not f.done():
                        c = f.emit()
                        if c is not None:
                            return c
            return None

        def drive(main_gen, ratio):
            debt = 2000.0
            for w in main_gen:
                debt += w * ratio
                while debt > 0:
                    c = pull()
                    if c is None:
                        break
                    debt -= c

        def drain_fillers(sel=None):
            while True:
                got = False
                for f in filler_q:
                    if sel is not None and f not in sel:
                        continue
                    if not f.done():
                        c = f.emit()
                        got = True
                        break
                if not got:
                    break

        # ---------------- phase 1 blocks 0..2 ----------------
        for nb in range(3):
            if nb >= 1:
                hs_tiles[nb + 1] = hs_load(nb + 1)
            run(qk_tile_gen(nb, 4))
            run(qk_tile_gen(nb, 5))
            for st2 in range(BW // 128):
                run(v_tile_gen(nb, st2))
            for mt in range(NQ):
                run(qk_tile_gen(nb, mt))

        # ---------------- block 3 head/k parts, then streamed ph2 ----------
        run(qk_tile_gen(3, 4))
        run(qk_tile_gen(3, 5))
        run(qk_tile_gen(3, 0))

        RATIO = 1.5

        # window 0: scores head 0; fillers = remaining projections
        f_q1 = GenF(qk_tile_gen(3, 1))
        f_q2 = GenF(qk_tile_gen(3, 2))
        f_v30 = GenF(v_tile_gen(3, 0))
        f_v31 = GenF(v_tile_gen(3, 1))
        f_q3 = GenF(qk_tile_gen(3, 3))
        f_v32 = GenF(v_tile_gen(3, 2))
        f_v33 = GenF(v_tile_gen(3, 3))
        ph1_tail = [f_q1, f_q2, f_v30, f_v31, f_q3, f_v32, f_v33]
        filler_q.extend(ph1_tail)
        filler_q.append(PVF(0))

        drive(scores_stream(0, ptp), RATIO)
        filler_q.append(PVF(1))
        drive(scores_stream(1, ptp), RATIO)

        # all phase-1 consumers must be emitted before releasing its SBUF
        drain_fillers(sel=ph1_tail)
        ph1.close()
        ptp_x = ExitStack()
        ptpx = ptp_x.enter_context(tc.tile_pool(name="p2ptx", bufs=1))
        wop = ptp_x.enter_context(tc.tile_pool(name="p3wo", bufs=1))
        ostg_stk = ExitStack()
        ostg = ostg_stk.enter_context(tc.tile_pool(name="p3stg", bufs=2))
        wo_sb = wop.tile([128, NQ, H], fmm)
        # gpsimd SW-DGE ring: keeps the ScalarE and SP queues free for
        # exps / transposes (in-order queues head-of-line block otherwise)
        for a in range(NQ):
            nc.gpsimd.dma_start(out=wo_sb[:, a, :], in_=wo_d[:, a, :])

        filler_q.append(PVF(2))
        drive(scores_stream(2, ptpx), RATIO)

        # head 3 reuses the first main pT buffer: pv0 must be fully emitted
        drain_fillers(sel=[f for f in filler_q if isinstance(f, PVF) and f.a == 0])
        ostg_tiles = [None] * (S // BW)
        filler_q.append(PVF(3))
        for ns in range(S // BW):
            filler_q.append(OF(ns, wo_sb, ostg_tiles))
        drive(scores_stream(3, ptp), RATIO)
        scalar_free[0] = True

        # drain: pv leftovers + o_proj, ready-gated then forced
        while True:
            c = pull()
            if c is None:
                c = pull(force=True)
                if c is None:
                    break

        ostg_stk.close()
        ptp_x.close()
        ph2.close()

    nc.finalize()
    return nc


def _rope_tables():
    inv_freq = 1.0 / (10000.0 ** (np.arange(0, D, 2, dtype=np.float32) / D))
    t = np.arange(S, dtype=np.float32)[:, None]
    freqs = t * inv_freq[None, :]          # [S, 64]
    cos = np.cos(freqs).astype(np.float32)  # [S, 64]
    sin = np.sin(freqs).astype(np.float32)
    mdt = np.dtype(MM_DT)
    cosf = np.concatenate([cos, cos], axis=1).T.astype(mdt)    # [128, S]
    sins = np.concatenate([-sin, sin], axis=1).T.astype(mdt)   # [128, S]
    return np.ascontiguousarray(cosf), np.ascontiguousarray(sins)


def _prep_in_maps(hidden_states, Wq, Wk, Wv, Wo):
    mdt = np.dtype(MM_DT)
    cosf, sins = _rope_tables()
    mask = np.triu(np.ones((128, 128), dtype=mdt))  # [j, q]: 1 if j <= q

    hsT_blocks = []
    for b in range(B):
        hsT = hidden_states[b].T  # [H, S]
        blk = np.ascontiguousarray(
            hsT.reshape(HC, 128, NB, BW).transpose(2, 1, 0, 3).astype(mdt)
        )  # [NB, 128, HC, BW]
        hsT_blocks.append(blk)

    in_maps = []
    for i in range(8):
        b, g = i // 4, i % 4
        wq = np.ascontiguousarray(
            Wq[512 * g : 512 * (g + 1), :].reshape(512, HC, 128).transpose(2, 1, 0).astype(mdt)
        )
        wk = np.ascontiguousarray(
            Wk[256 * g : 256 * (g + 1), :].reshape(256, HC, 128).transpose(2, 1, 0).astype(mdt)
        )
        wv = np.ascontiguousarray(
            Wv[256 * g : 256 * (g + 1), :].reshape(256, HC, 128).transpose(2, 1, 0).astype(mdt)
        )
        wo = np.ascontiguousarray(
            Wo[:, 512 * g : 512 * (g + 1)].reshape(H, NQ, 128).transpose(2, 1, 0).astype(mdt)
        )
        in_maps.append(
            {
                "hsT": hsT_blocks[b],
                "wq": wq,
                "wk": wk,
                "wv": wv,
                "wo": wo,
                "cosf": cosf,
                "sins": sins,
                "mask": mask,
            }
        )
    return in_maps


def _run(in_maps, **kwargs):
    from concourse.bass_utils import run_bass_kernel_spmd

    if "prog" not in _CACHE:
        _CACHE["prog"] = _build_program()
    nc = _CACHE["prog"]
    return run_bass_kernel_spmd(nc, in_maps, core_ids=list(range(8)), **kwargs)


def _gather(results):
    out = np.empty((B, S, H), dtype=np.float32)
    for b in range(B):
        acc = results[4 * b + 0]["outT"].astype(np.float32)
        for g in range(1, 4):
            acc += results[4 * b + g]["outT"].astype(np.float32)
        out[b] = acc.T
    return out


def kernel(hidden_states, Wq, Wk, Wv, Wo):
    hidden_states = np.asarray(hidden_states, dtype=np.float32)
    Wq = np.asarray(Wq, dtype=np.float32)
    Wk = np.asarray(Wk, dtype=np.float32)
    Wv = np.asarray(Wv, dtype=np.float32)
    Wo = np.asarray(Wo, dtype=np.float32)
    in_maps = _prep_in_maps(hidden_states, Wq, Wk, Wv, Wo)
    res = _run(in_maps)
    return _gather(res.results)


# revision 35
# speedup vs baseline: 1.1977x; 1.0119x over previous
"""Trainium2 Bass kernel for GQA attention block (nn_Attention_20272245637793).

Reference computation (B=2, S=2048, H=2048, 16 q heads / 8 kv heads, D=128):
    q = hs @ Wq.T ; k = hs @ Wk.T ; v = hs @ Wv.T
    rope(q), rope(k); causal softmax(q k^T / sqrt(D)) @ v ; out @ Wo.T

Sharding (8 cores): core i = (b, g) with b = i // 4 (data-parallel over
batch), g = i % 4 (tensor-parallel over kv-head groups; kv heads {2g, 2g+1},
q heads {4g..4g+3}).  Each core computes 1/8 of every GEMM and a partial
o_proj over its 512 head-dims; the host sums the 4 partials per batch.

v3 changes over v2 (trace-driven; v2 = 280.5us, PE busy 84%):
  * Phase 2 was ScalarE-exp-bound: 96 ACTIVATEs x (cols*0.833ns + ~274ns
    fixed).  Scores psum tiles are now [128,1536] (3 banks) and the exp'd
    stripes are packed contiguously, so one ACTIVATE covers 1536 pT columns
    across stripe boundaries: 48 ACTIVATEs total (exp bound 84us -> 75us).
  * Phase 2+3 emission is one continuous stream: scores heads 0..3 is the
    main stream; a ready-gated ordered filler queue (leftover phase-1
    projections, per-stripe-eager PV for every head, o_proj blocks gated on
    PV/transposes) is paced by column debt, eliminating the window seams.
  * PSUM: s_ps 2x[128,1536] for scores/exp + mm_ps 2x[128,512] shared by
    projections, PV pairs and o_proj (8 banks exactly).
  * pT triple buffering (2-buf pool + 1-buf pool opened after phase-1 SBUF
    is released) so head a+1 scores start while head a-1 PV drains.
  * Startup DMA: wv/wq moved to the sync ring behind hs block 0 (ahead of
    hs block 1) so the weight ring only carries wk+trig early; cos/sin are
    loaded in a [0:512] slab first; exp act-table is preloaded at t=0.
  * o_proj psum->SBUF copies alternate ScalarE/DVE; outT stores go on the
    scalar ring; final stores are per-128-column so the end drain is short.

Built on bacc.Bacc: TRN2 instructions can carry at most ONE semaphore wait;
Bacc.compile() legalizes multi-wait instructions.
"""

import sys

sys.path.insert(0, "/opt/trn_rl_repo")

import numpy as np
from bisect import bisect_right
from contextlib import ExitStack

B = 2
S = 2048
H = 2048
D = 128
NQ = 4          # q heads per core
NKVL = 2        # kv heads per core
HC = H // 128   # 16 h-chunks (contraction)
NB = 4          # hs^T column blocks of 512 for projections
BW = S // NB    # 512
ST = S // 128   # 16 s-tiles / k-chunks / q-tiles
SCALE = 1.0 / np.sqrt(D)

# stripe c of the exp'd transposed scores covers q in [128c, S); offsets of
# the stripes packed into one [128, PT_TOTAL] sbuf tile
STRIPE_LEN = [S - 128 * c for c in range(ST)]
STRIPE_OFF = np.concatenate([[0], np.cumsum(STRIPE_LEN)]).tolist()
PT_TOTAL = STRIPE_OFF[-1]  # 17408

CH = 1536       # exp chunk width (3 psum banks)

MM_DT = "float16"

_CACHE = {}


def _build_program():
    import concourse.tile as tile
    from concourse import bacc, mybir

    f32 = mybir.dt.float32
    fmm = getattr(mybir.dt, MM_DT)
    nc = bacc.Bacc()

    hsT_d = nc.declare_dram_parameter("hsT", [NB, 128, HC, BW], fmm, isOutput=False)
    wq_d = nc.declare_dram_parameter("wq", [128, HC, 128 * NQ], fmm, isOutput=False)
    wk_d = nc.declare_dram_parameter("wk", [128, HC, 128 * NKVL], fmm, isOutput=False)
    wv_d = nc.declare_dram_parameter("wv", [128, HC, 128 * NKVL], fmm, isOutput=False)
    wo_d = nc.declare_dram_parameter("wo", [128, NQ, H], fmm, isOutput=False)
    cos_d = nc.declare_dram_parameter("cosf", [128, S], fmm, isOutput=False)
    sin_d = nc.declare_dram_parameter("sins", [128, S], fmm, isOutput=False)
    mask_d = nc.declare_dram_parameter("mask", [128, 128], fmm, isOutput=False)
    outT_d = nc.declare_dram_parameter("outT", [H, S], fmm, isOutput=True)

    with tile.TileContext(nc) as tc, ExitStack() as top:
        glob = top.enter_context(tc.tile_pool(name="glob", bufs=1))
        qrot = glob.tile([128, NQ, S], fmm)      # q^T, rope'd, per head
        krot = glob.tile([128, NKVL, S], fmm)    # k^T, rope'd, per kv head
        vaug = glob.tile([128, NKVL, ST, 132], fmm)  # v chunks + ones col @128
        attnT = glob.tile([128, NQ, ST, 128], fmm)  # attention out, transposed
        mask_sb = glob.tile([128, 128], fmm)
        warm = glob.tile([128, 1], f32)

        nc.vector.memset(vaug[:, :, :, 128:129], 1.0)
        nc.vector.memset(warm, 0.0)

        # ---------------- pools (stack allocator: long-lived first) --------
        ph2 = ExitStack()
        ptp = ph2.enter_context(tc.tile_pool(name="p2pt", bufs=2))
        stg = ph2.enter_context(tc.tile_pool(name="p2stg", bufs=6))
        smal = ph2.enter_context(tc.tile_pool(name="p2small", bufs=4))
        s_ps = ph2.enter_context(tc.tile_pool(name="p2sps", bufs=2, space="PSUM"))
        mm_ps = ph2.enter_context(tc.tile_pool(name="p2mm", bufs=2, space="PSUM"))

        ph1 = ExitStack()
        consts = ph1.enter_context(tc.tile_pool(name="p1const", bufs=1))
        hsp = ph1.enter_context(tc.tile_pool(name="p1hs", bufs=2))
        ropep = ph1.enter_context(tc.tile_pool(name="p1rope", bufs=3))

        # ---------------- phase 1 emission helpers ----------------
        def hs_load(nb, split_first=False):
            t = hsp.tile([128, HC, BW], fmm, name=f"hs_{nb}", tag="hs")
            if split_first:
                # block 0 in three slabs: chunk 0 alone so the very first
                # matmul starts immediately, then two large slabs that land
                # on early-starting DMA engines
                nc.sync.dma_start(out=t[:, 0:1, :], in_=hsT_d[nb, :, 0:1, :])
                nc.sync.dma_start(out=t[:, 1:8, :], in_=hsT_d[nb, :, 1:8, :])
                nc.sync.dma_start(out=t[:, 8:16, :], in_=hsT_d[nb, :, 8:16, :])
                return t
            nc.sync.dma_start(out=t[:, 0:4, :], in_=hsT_d[nb, :, 0:4, :])
            for c4 in range(4, HC, 4):
                nc.sync.dma_start(out=t[:, c4 : c4 + 4, :], in_=hsT_d[nb, :, c4 : c4 + 4, :])
            return t

        # startup loads.  sync ring pairs hs0 chunks with wk chunks in the
        # exact k-gen consumption order; scalar ring carries trig + wv + wq.
        wq_sb = consts.tile([128, HC, 128 * NQ], fmm)
        wk_sb = consts.tile([128, HC, 128 * NKVL], fmm)
        wv_sb = consts.tile([128, HC, 128 * NKVL], fmm)
        cos_sb = consts.tile([128, S], fmm)
        sin_sb = consts.tile([128, S], fmm)
        hs_tiles = [None] * NB
        hs_tiles[0] = hs_load(0, split_first=True)
        nc.scalar.dma_start(out=wk_sb[:, 0:1, :], in_=wk_d[:, 0:1, :])
        nc.scalar.dma_start(out=wk_sb[:, 1:8, :], in_=wk_d[:, 1:8, :])
        nc.scalar.dma_start(out=wk_sb[:, 8:16, :], in_=wk_d[:, 8:16, :])
        nc.scalar.dma_start(out=cos_sb[:, 0:512], in_=cos_d[:, 0:512])
        nc.scalar.dma_start(out=sin_sb[:, 0:512], in_=sin_d[:, 0:512])
        # preload the exp act-table (~2.7us) while the DMAs stream; placed
        # after the early scalar-ring issues so it doesn't delay them
        nc.scalar.activation(warm, warm, mybir.ActivationFunctionType.Exp)
        for c4 in range(0, HC, 4):
            nc.sync.dma_start(out=wv_sb[:, c4 : c4 + 4, :], in_=wv_d[:, c4 : c4 + 4, :])
        for c4 in range(0, HC, 4):
            nc.sync.dma_start(out=wq_sb[:, c4 : c4 + 4, :], in_=wq_d[:, c4 : c4 + 4, :])
        hs_tiles[1] = hs_load(1)
        nc.scalar.dma_start(out=cos_sb[:, 512:S], in_=cos_d[:, 512:S])
        nc.scalar.dma_start(out=sin_sb[:, 512:S], in_=sin_d[:, 512:S])
        nc.scalar.dma_start(out=mask_sb, in_=mask_d[:, :])

        def qk_tile_gen(nb, mt):
            """mt 0..3 = q heads, 4..5 = k heads. Yields cols after each mm."""
            n0 = nb * BW
            hs_t = hs_tiles[nb]
            ps = mm_ps.tile([128, BW], f32, tag="mmps")
            if mt < NQ:
                w_sb, mo = wq_sb, mt
            else:
                w_sb, mo = wk_sb, mt - NQ
            for c in range(HC):
                nc.tensor.matmul(
                    ps,
                    w_sb[:, c, 128 * mo : 128 * mo + 128],
                    hs_t[:, c, :],
                    start=(c == 0),
                    stop=(c == HC - 1),
                )
                yield BW
            if mt < NQ:
                dest = qrot[:, mt, n0 : n0 + BW]
            else:
                dest = krot[:, mt - NQ, n0 : n0 + BW]
            # rope: dest = ps * cos + swap_halves(ps) * (+/-)sin
            t_t = ropep.tile([128, BW], f32, tag="ropet")
            u_t = ropep.tile([128, BW], f32, tag="ropeu")
            nc.vector.tensor_mul(t_t, ps, cos_sb[:, n0 : n0 + BW])
            nc.vector.tensor_mul(u_t[0:64, :], ps[64:128, :], sin_sb[0:64, n0 : n0 + BW])
            nc.vector.tensor_mul(u_t[64:128, :], ps[0:64, :], sin_sb[64:128, n0 : n0 + BW])
            nc.vector.tensor_add(dest, t_t, u_t)

        def v_tile_gen(nb, st2):
            st = (BW // 128) * nb + st2
            hs_t = hs_tiles[nb]
            psw = mm_ps.tile([128, BW], f32, tag="mmps")
            ps = psw[:, 0 : 128 * NKVL]
            for c in range(HC):
                nc.tensor.matmul(
                    ps,
                    hs_t[:, c, 128 * st2 : 128 * st2 + 128],
                    wv_sb[:, c, :],
                    start=(c == 0),
                    stop=(c == HC - 1),
                )
                yield 128 * NKVL
            # single strided cast: [128, 2, 128] psum -> vaug[:, :, st, 0:128]
            nc.vector.tensor_copy(
                vaug[:, :, st, 0:128],
                ps.rearrange("p (kv d) -> p kv d", kv=NKVL),
            )

        def run(gen):
            for _ in gen:
                pass

        # ---------------- phase 2: scores main stream ----------------
        pT_tiles = [None] * NQ
        stripes_done = [0] * NQ   # fully exp'd + masked stripes per head
        pv_tiles = [0] * NQ       # PV output tiles emitted per head

        def scores_stream(a, pool):
            """Main-stream generator for head a: yields cols after each
            scores sub-matmul.  Exp chunks of CH pT columns, stripe packing,
            per-bank start/stop flags, masks + stripe bookkeeping."""
            kv = a // 2
            pT = pool.tile([128, PT_TOTAL], fmm, tag="pT")
            pT_tiles[a] = pT
            pos = 0
            masked = 0  # stripes masked so far
            while pos < PT_TOTAL:
                clen = min(CH, PT_TOTAL - pos)
                ps = s_ps.tile([128, CH], f32, tag="sps")
                seg = pos
                while seg < pos + clen:
                    c = bisect_right(STRIPE_OFF, seg) - 1
                    send = STRIPE_OFF[c + 1]
                    boff = seg - pos
                    bank_end = pos + (boff // 512 + 1) * 512
                    end = min(send, bank_end)
                    w = end - seg
                    qcol = 128 * c + (seg - STRIPE_OFF[c])
                    first_in_bank = (boff % 512) == 0
                    last_in_bank = end == bank_end
                    nc.tensor.matmul(
                        ps[:, boff : boff + w],
                        krot[:, kv, 128 * c : 128 * c + 128],
                        qrot[:, a, qcol : qcol + w],
                        start=first_in_bank,
                        stop=last_in_bank,
                        skip_group_check=not (first_in_bank and last_in_bank),
                    )
                    yield w
                    seg = end
                nc.scalar.activation(
                    pT[:, pos : pos + clen],
                    ps[:, 0:clen],
                    mybir.ActivationFunctionType.Exp,
                    scale=float(SCALE),
                )
                pos += clen
                # masks for newly covered diagonal blocks
                while masked < ST and STRIPE_OFF[masked] + 128 <= pos:
                    off = STRIPE_OFF[masked]
                    nc.vector.tensor_mul(
                        pT[:, off : off + 128], pT[:, off : off + 128], mask_sb
                    )
                    masked += 1
                # stripe completion (exp coverage + mask emitted)
                nd = bisect_right(STRIPE_OFF, pos) - 1
                stripes_done[a] = min(nd, masked)

        # ---------------- PV ----------------
        pv_stage = [None] * NQ

        def pv_pair_gen(a, t0):
            """PV + normalize for tiles t0, t0+1 sharing one PSUM bank:
            chain t0 at cols [0:129], t0+1 at [132:261].  The start=True
            matmul of chain t0 zeroes the whole 2KB bank, so chain t0+1
            accumulates with start=False throughout.  Two pairs share one
            [128,512] stage; the pair at t0%4==2 emits a single batched
            XBAR transpose covering the 4 tiles of o_proj block t0//4."""
            kv = a // 2
            pT = pT_tiles[a]
            t1 = t0 + 1
            po = mm_ps.tile([128, BW], f32, tag="mmps")
            for c in range(t1 + 1):
                if c <= t0:
                    lhsT = pT[
                        :,
                        STRIPE_OFF[c] + 128 * (t0 - c) : STRIPE_OFF[c] + 128 * (t0 - c) + 128,
                    ]
                    nc.tensor.matmul(
                        po[:, 0:129],
                        lhsT,
                        vaug[:, kv, c, 0:129],
                        start=(c == 0),
                        stop=(c == t0),
                        skip_group_check=True,
                    )
                    yield 258
                lhsT = pT[
                    :,
                    STRIPE_OFF[c] + 128 * (t1 - c) : STRIPE_OFF[c] + 128 * (t1 - c) + 128,
                ]
                nc.tensor.matmul(
                    po[:, 132:261],
                    lhsT,
                    vaug[:, kv, c, 0:129],
                    start=False,
                    stop=(c == t1),
                    skip_group_check=True,
                )
            yield 129
            if t0 % 4 == 0:
                pv_stage[a] = stg.tile([128, 512], fmm, name=f"stg_{a}_{t0}", tag="stage")
            stage = pv_stage[a]
            so = 256 * ((t0 % 4) // 2)
            for j, st2 in ((0, 0), (132, 1)):
                r = smal.tile([128, 1], f32, tag="recip")
                nc.vector.reciprocal(r, po[:, j + 128 : j + 129])
                nc.vector.tensor_scalar_mul(
                    stage[:, so + 128 * st2 : so + 128 * st2 + 128], po[:, j : j + 128], r
                )
            if t0 % 4 == 2:
                eng = nc.scalar if (scalar_free[0] and t0 % 8 == 6) else nc.sync
                eng.dma_start(
                    out=attnT[:, a, t0 - 2 : t0 + 2, :], in_=stage, transpose=True
                )
            pv_tiles[a] = t1 + 1

        # ---------------- o_proj ----------------
        outT_v = outT_d.rearrange("(m p) s -> p m s", p=128)
        o_copy_flip = [0]
        scalar_free = [False]  # True once all exps are emitted
        o_sps = {"tile": None, "k": 0}

        def o_mt_step(ns, mt, wo_sb, ostg_tiles):
            if mt == 0:
                ostg_tiles[ns] = ostg.tile(
                    [128, H // 128, BW], fmm, name=f"ostg_{ns}", tag="ostg"
                )
            ot = ostg_tiles[ns]
            if scalar_free[0] and pv_tiles[3] >= ST:
                # exps + pv done: rotate o_proj psum through all 8 banks
                # (freed scores banks + mm banks) so copy latency never
                # gates the matmuls
                k = o_sps["k"] % 4
                o_sps["k"] += 1
                if k == 3:
                    ps = mm_ps.tile([128, BW], f32, tag="mmps")
                else:
                    if k == 0:
                        o_sps["tile"] = s_ps.tile([128, CH], f32, name="osps", tag="sps")
                    ps = o_sps["tile"][:, 512 * k : 512 * k + 512]
            else:
                ps = mm_ps.tile([128, BW], f32, tag="mmps")
            for a in range(NQ):
                nc.tensor.matmul(
                    ps,
                    wo_sb[:, a, 128 * mt : 128 * mt + 128],
                    attnT[:, a, 4 * ns : 4 * ns + 4, :].rearrange("p t d -> p (t d)"),
                    start=(a == 0),
                    stop=(a == NQ - 1),
                )
            # ScalarE must stay exp-only until the last exp is emitted:
            # anything queued ahead of an exp head-of-line blocks it
            if scalar_free[0] and o_copy_flip[0] == 0:
                nc.scalar.copy(ot[:, mt, :], ps)
            else:
                nc.vector.tensor_copy(ot[:, mt, :], ps)
            o_copy_flip[0] ^= 1
            fine = ns == S // BW - 1 and mt >= 12
            step = 1 if fine else 2
            if (mt + 1) % step == 0:
                # the tail stores ride the scalar HW-DGE ring (ScalarE is
                # idle then and HW-DGE drains faster than gpsimd SW-DGE)
                eng = nc.scalar if fine else nc.gpsimd
                eng.dma_start(
                    out=outT_v[:, mt - step + 1 : mt + 1, BW * ns : BW * ns + BW],
                    in_=ot[:, mt - step + 1 : mt + 1, :],
                )
            return NQ * BW

        # ---------------- filler queue ----------------
        class GenF:
            """Wraps a generator yielding cost units; always ready."""
            def __init__(self, gen):
                self.gen = gen
                self.fin = False
            def ready(self):
                return True
            def emit(self):
                try:
                    return next(self.gen)
                except StopIteration:
                    self.fin = True
                    return None
            def done(self):
                return self.fin

        class PVF:
            def __init__(self, a):
                self.a = a
                self.t0 = 0
                self.cur = None
            def ready(self):
                if self.cur is not None:
                    return True
                return stripes_done[self.a] >= self.t0 + 2
            def emit(self):
                if self.cur is None:
                    self.cur = pv_pair_gen(self.a, self.t0)
                try:
                    return next(self.cur)
                except StopIteration:
                    self.cur = None
                    self.t0 += 2
                    if self.t0 >= ST:
                        return None
                    if stripes_done[self.a] >= self.t0 + 2:
                        self.cur = pv_pair_gen(self.a, self.t0)
                        return next(self.cur)
                    return None
            def done(self):
                return self.t0 >= ST and self.cur is None

        O_LAG = 4

        class OF:
            def __init__(self, ns, wo_sb, ostg_tiles):
                self.ns = ns
                self.mt = 0
                self.wo_sb = wo_sb
                self.ostg_tiles = ostg_tiles
            def ready(self):
                need = 4 * self.ns + 4
                for a in range(3):
                    if pv_tiles[a] < need:
                        return False
                return pv_tiles[3] >= min(ST, need + O_LAG)
            def emit(self):
                cost = o_mt_step(self.ns, self.mt, self.wo_sb, self.ostg_tiles)
                self.mt += 1
                return cost
            def done(self):
                return self.mt >= H // 128

        filler_q = []

        def pull(force=False):
            """Emit one unit from the first ready filler; returns cost or
            None.  force: emit the first unfinished filler even if gated."""
            for f in filler_q:
                if not f.done() and f.ready():
                    c = f.emit()
                    if c is not None:
                        return c
            if force:
                for f in filler_q:
                    if not f.done():
                        c = f.emit()
                        if c is not None:
                            return c
            return None

        def drive(main_gen, ratio):
            debt = 2000.0
            for w in main_gen:
                debt += w * ratio
                while debt > 0:
                    c = pull()
                    if c is None:
                        break
                    debt -= c

        def drain_fillers(sel=None):
            while True:
                got = False
                for f in filler_q:
                    if sel is not None and f not in sel:
                        continue
                    if not f.done():
                        c = f.emit()
                        got = True
                        break
                if not got:
                    break

        # ---------------- phase 1 blocks 0..2 ----------------
        for nb in range(3):
            if nb >= 1:
                hs_tiles[nb + 1] = hs_load(nb + 1)
            run(qk_tile_gen(nb, 4))
            run(qk_tile_gen(nb, 5))
            for st2 in range(BW // 128):
                run(v_tile_gen(nb, st2))
            for mt in range(NQ):
                run(qk_tile_gen(nb, mt))

        # ---------------- block 3 head/k parts, then streamed ph2 ----------
        run(qk_tile_gen(3, 4))
        run(qk_tile_gen(3, 5))
        run(qk_tile_gen(3, 0))

        RATIO = 1.5

        # window 0: scores head 0; fillers = remaining projections
        f_q1 = GenF(qk_tile_gen(3, 1))
        f_q2 = GenF(qk_tile_gen(3, 2))
        f_v30 = GenF(v_tile_gen(3, 0))
        f_v31 = GenF(v_tile_gen(3, 1))
        f_q3 = GenF(qk_tile_gen(3, 3))
        f_v32 = GenF(v_tile_gen(3, 2))
        f_v33 = GenF(v_tile_gen(3, 3))
        ph1_tail = [f_q1, f_q2, f_v30, f_v31, f_q3, f_v32, f_v33]
        filler_q.extend(ph1_tail)
        filler_q.append(PVF(0))

        drive(scores_stream(0, ptp), RATIO)
        filler_q.append(PVF(1))
        drive(scores_stream(1, ptp), RATIO)

        # all phase-1 consumers must be emitted before releasing its SBUF
        drain_fillers(sel=ph1_tail)
        ph1.close()
        ptp_x = ExitStack()
        ptpx = ptp_x.enter_context(tc.tile_pool(name="p2ptx", bufs=1))
        wop = ptp_x.enter_context(tc.tile_pool(name="p3wo", bufs=1))
        ostg_stk = ExitStack()
        ostg = ostg_stk.enter_context(tc.tile_pool(name="p3stg", bufs=2))
        wo_sb = wop.tile([128, NQ, H], fmm)
        # gpsimd SW-DGE ring: keeps the ScalarE and SP queues free for
        # exps / transposes (in-order queues head-of-line block otherwise)
        for a in range(NQ):
            nc.gpsimd.dma_start(out=wo_sb[:, a, :], in_=wo_d[:, a, :])

        filler_q.append(PVF(2))
        drive(scores_stream(2, ptpx), RATIO)

        # head 3 reuses the first main pT buffer: pv0 must be fully emitted
        drain_fillers(sel=[f for f in filler_q if isinstance(f, PVF) and f.a == 0])
        ostg_tiles = [None] * (S // BW)
        filler_q.append(PVF(3))
        for ns in range(S // BW):
            filler_q.append(OF(ns, wo_sb, ostg_tiles))
        drive(scores_stream(3, ptp), RATIO)
        scalar_free[0] = True

        # drain: pv leftovers + o_proj, ready-gated then forced
        while True:
            c = pull()
            if c is None:
                c = pull(force=True)
                if c is None:
                    break

        ostg_stk.close()
        ptp_x.close()
        ph2.close()

    nc.finalize()
    return nc


def _rope_tables():
    inv_freq = 1.0 / (10000.0 ** (np.arange(0, D, 2, dtype=np.float32) / D))
    t = np.arange(S, dtype=np.float32)[:, None]
    freqs = t * inv_freq[None, :]          # [S, 64]
    cos = np.cos(freqs).astype(np.float32)  # [S, 64]
    sin = np.sin(freqs).astype(np.float32)
    mdt = np.dtype(MM_DT)
    cosf = np.concatenate([cos, cos], axis=1).T.astype(mdt)    # [128, S]
    sins = np.concatenate([-sin, sin], axis=1).T.astype(mdt)   # [128, S]
    return np.ascontiguousarray(cosf), np.ascontiguousarray(sins)


def _prep_in_maps(hidden_states, Wq, Wk, Wv, Wo):
    mdt = np.dtype(MM_DT)
    cosf, sins = _rope_tables()
    mask = np.triu(np.ones((128, 128), dtype=mdt))  # [j, q]: 1 if j <= q

    hsT_blocks = []
    for b in range(B):
        hsT = hidden_states[b].T  # [H, S]
        blk = np.ascontiguousarray(
            hsT.reshape(HC, 128, NB, BW).transpose(2, 1, 0, 3).astype(mdt)
        )  # [NB, 128, HC, BW]
        hsT_blocks.append(blk)

    in_maps = []
    for i in range(8):
        b, g = i // 4, i % 4
        wq = np.ascontiguousarray(
            Wq[512 * g : 512 * (g + 1), :].reshape(512, HC, 128).transpose(2, 1, 0).astype(mdt)
        )
        wk = np.ascontiguousarray(
            Wk[256 * g : 256 * (g + 1), :].reshape(256, HC, 128).transpose(2, 1, 0).astype(mdt)
        )
        wv = np.ascontiguousarray(
            Wv[256 * g : 256 * (g + 1), :].reshape(256, HC, 128).transpose(2, 1, 0).astype(mdt)
        )
        wo = np.ascontiguousarray(
            Wo[:, 512 * g : 512 * (g + 1)].reshape(H, NQ, 128).transpose(2, 1, 0).astype(mdt)
        )
        in_maps.append(
            {
                "hsT": hsT_blocks[b],
                "wq": wq,
                "wk": wk,
                "wv": wv,
                "wo": wo,
                "cosf": cosf,
                "sins": sins,
                "mask": mask,
            }
        )
    return in_maps


def _run(in_maps, **kwargs):
    from concourse.bass_utils import run_bass_kernel_spmd

    if "prog" not in _CACHE:
        _CACHE["prog"] = _build_program()
    nc = _CACHE["prog"]
    return run_bass_kernel_spmd(nc, in_maps, core_ids=list(range(8)), **kwargs)


def _gather(results):
    out = np.empty((B, S, H), dtype=np.float32)
    for b in range(B):
        acc = results[4 * b + 0]["outT"].astype(np.float32)
        for g in range(1, 4):
            acc += results[4 * b + g]["outT"].astype(np.float32)
        out[b] = acc.T
    return out


def kernel(hidden_states, Wq, Wk, Wv, Wo):
    hidden_states = np.asarray(hidden_states, dtype=np.float32)
    Wq = np.asarray(Wq, dtype=np.float32)
    Wk = np.asarray(Wk, dtype=np.float32)
    Wv = np.asarray(Wv, dtype=np.float32)
    Wo = np.asarray(Wo, dtype=np.float32)
    in_maps = _prep_in_maps(hidden_states, Wq, Wk, Wv, Wo)
    res = _run(in_maps)
    return _gather(res.results)
